# revision 14
# baseline (speedup 1.0000x reference)
"""Trainium2 Bass kernel: batched 1000-step controller rollout + LQR-style cost.

Problem: B=1024 independent controllers, each rolling z_{t+1} = MLP([z_t, u_t])
for Ts=1000 steps with u_t = -controller . z_t, then
cost_b = mean_t (z^T Q z + R u^2).

Sharding: data-parallel over batch; 128 controllers per core on 8 cores.

State S[9,128] = [z(4); y(4); ones(1)] with y_k = -ctrl_k * z_k, so
u = sum_k y_k is folded into the layer-1 contraction and b1 rides the ones row
(lhsT rows = [W1[0:4]; W1[4] x4; b1]). All math fp32 (bf16 state diverges over
1000 steps; float32r matmuls are numerically broken on TRN2). fp32 weight
loads stream at a fixed ~1.2GHz (2 passes x M columns), so each h-matmul pays
a ~213ns LDWEIGHTS toll — a single 128-wide group pays it once per half per
step (two 64-wide chains would pay it twice). Latency is hidden by splitting
the post-h pipeline per hidden half: tanh(h0) and the first z-matmul run while
mm_h1 / tanh(h1) are still in flight (separate PSUM banks per half make the
dependencies unambiguous).

Per step:
  psum_h{0,1}  = W1aug_half^T @ S             (2 matmuls, K=9, N=128)
  h{0,1}       = tanh(psum_h{0,1})            (2 ACT ops, bias-free)
  psum_z[8,128]= W2dup_0^T @ h0 + W2dup_1^T @ h1  (2 matmuls, K=128, M=8: W2
                                               cols duplicated so psum_z = [z;z])
  S'[0:8] = (psum_z + b2dup) * [ones(4); negctrl]   (1 DVE scalar_tensor_tensor)
  DMA S'[0:8] -> DRAM trajectory bounce       (off critical path)

Cost pass: DMA 16-state chunks back as [128,128] tiles; q = BD^T @ chunk with
BD block-diag([sym(Q); R*ones(4,4)])/(Ts+1); tp = chunk * q (DVE); accumulate
ones^T @ tp into psum_cost[1,128] across chunks.
"""

import numpy as np
from contextlib import ExitStack

import concourse.bass as bass
import concourse.bacc as bacc
import concourse.tile as tile
from concourse import mybir
from concourse._compat import get_trn_type
from concourse.bass_utils import run_bass_kernel_spmd

F32 = mybir.dt.float32
AFT = mybir.ActivationFunctionType
ALU = mybir.AluOpType

B = 1024
H = 256
SD = 2
DIM_X = 4
N_CORES = 8
BS = B // N_CORES   # 128 batch columns per core
N_SBUF = 3           # persistent S tiles (rotation depth)
STATES_PER_CHUNK = 16


def build_program(n_steps: int):
    """Build the Bass/Tile program for one core (SPMD across 8)."""
    n_states = n_steps + 1
    n_chunks = (n_states + STATES_PER_CHUNK - 1) // STATES_PER_CHUNK
    n_slots = n_chunks * STATES_PER_CHUNK

    nc = bacc.Bacc(
        get_trn_type() or "TRN2",
        target_bir_lowering=False,
        debug=False,
        num_devices=N_CORES,
    )

    d_w1a0 = nc.dram_tensor("W1a0", [9, 128], F32, kind="ExternalInput")
    d_w1a1 = nc.dram_tensor("W1a1", [9, 128], F32, kind="ExternalInput")
    d_w2d0 = nc.dram_tensor("W2d0", [128, 8], F32, kind="ExternalInput")
    d_w2d1 = nc.dram_tensor("W2d1", [128, 8], F32, kind="ExternalInput")
    d_b2d = nc.dram_tensor("b2d", [8, 1], F32, kind="ExternalInput")
    d_cmat = nc.dram_tensor("Cmat", [8, BS], F32, kind="ExternalInput")
    d_sinit = nc.dram_tensor("Sinit", [9, BS], F32, kind="ExternalInput")
    d_bd = nc.dram_tensor("BD", [128, 128], F32, kind="ExternalInput")
    d_cost = nc.dram_tensor("cost", [1, BS], F32, kind="ExternalOutput")
    d_traj = nc.dram_tensor("trajb", [n_slots, 8, BS], F32)  # Internal bounce

    with tile.TileContext(nc) as tc, ExitStack() as ctx:
        consts = ctx.enter_context(tc.tile_pool(name="consts", bufs=1))
        hpool = ctx.enter_context(tc.tile_pool(name="hpool", bufs=2))
        chpool = ctx.enter_context(tc.tile_pool(name="chpool", bufs=3))
        tpool = ctx.enter_context(tc.tile_pool(name="tpool", bufs=3))

        w1a0 = consts.tile([9, 128], F32)
        w1a1 = consts.tile([9, 128], F32)
        w2d0 = consts.tile([128, 8], F32)
        w2d1 = consts.tile([128, 8], F32)
        b2d = consts.tile([8, 1], F32)
        cmat = consts.tile([8, BS], F32)
        bd = consts.tile([128, 128], F32)
        ones_col = consts.tile([128, 1], F32)
        zpad = consts.tile([8, BS], F32)
        cost_sb = consts.tile([1, BS], F32)

        nc.sync.dma_start(out=w1a0[:], in_=d_w1a0[:])
        nc.sync.dma_start(out=w1a1[:], in_=d_w1a1[:])
        nc.sync.dma_start(out=w2d0[:], in_=d_w2d0[:])
        nc.sync.dma_start(out=w2d1[:], in_=d_w2d1[:])
        nc.sync.dma_start(out=b2d[:], in_=d_b2d[:])
        nc.sync.dma_start(out=cmat[:], in_=d_cmat[:])
        nc.sync.dma_start(out=bd[:], in_=d_bd[:])
        nc.vector.memset(ones_col[:], 1.0)
        nc.vector.memset(zpad[:], 0.0)

        # zero the padding slots of the trajectory so they contribute 0 cost
        for i in range(n_states, n_slots):
            nc.sync.dma_start(out=d_traj[i], in_=zpad[:])

        # tiny dependency-free bf16 matmuls to keep the PE array active (and
        # its HAM-gated clock ramped) while real matmuls wait on tanh / stt
        fmm_w = consts.tile([1, 32], F32)
        fmm_x = consts.tile([1, 32], F32)
        nc.vector.memset(fmm_w[:], 0.0)
        nc.vector.memset(fmm_x[:], 0.0)

        # persistent state tiles; row 8 stays = 1.0 (written by the init DMA,
        # never touched by the per-step state write to rows 0:8)
        S_tiles = [
            consts.tile([9, BS], F32, name=f"S_{k}", tag=f"S_{k}")
            for k in range(N_SBUF)
        ]
        for k in range(N_SBUF):
            nc.sync.dma_start(out=S_tiles[k][:], in_=d_sinit[:])
        nc.sync.dma_start(out=d_traj[0], in_=S_tiles[0][0:8])

        with (
            tc.tile_pool(name="psumh0", bufs=2, space="PSUM") as psumh0,
            tc.tile_pool(name="psumh1", bufs=2, space="PSUM") as psumh1,
            tc.tile_pool(name="psumz", bufs=2, space="PSUM") as psumz,
            tc.tile_pool(name="psumf", bufs=1, space="PSUM") as psumf,
        ):
            fp = psumf.tile([32, 32], F32)

            def fillers(n):
                for _ in range(n):
                    nc.tensor.matmul(
                        fp[:], fmm_w[:], fmm_x[:], start=True, stop=True
                    )

            for t in range(n_steps):
                S = S_tiles[t % N_SBUF]
                Sn = S_tiles[(t + 1) % N_SBUF]
                ph0 = psumh0.tile([128, BS], F32, tag="ph0")
                ph1 = psumh1.tile([128, BS], F32, tag="ph1")
                nc.tensor.matmul(ph0[:], w1a0[:], S[:], start=True, stop=True)
                nc.tensor.matmul(ph1[:], w1a1[:], S[:], start=True, stop=True)
                h0 = hpool.tile([128, BS], F32, tag="h0")
                h1 = hpool.tile([128, BS], F32, tag="h1")
                nc.scalar.activation(h0[:], ph0[:], AFT.Tanh)
                nc.scalar.activation(h1[:], ph1[:], AFT.Tanh)
                pz = psumz.tile([8, BS], F32, tag="pz")
                nc.tensor.matmul(pz[:], w2d0[:], h0[:], start=True, stop=False)
                nc.tensor.matmul(pz[:], w2d1[:], h1[:], start=False, stop=True)
                fillers(5)
                nc.vector.scalar_tensor_tensor(
                    out=Sn[0:8], in0=pz[:], scalar=b2d[:], in1=cmat[:],
                    op0=ALU.add, op1=ALU.mult,
                )
                nc.sync.dma_start(out=d_traj[t + 1], in_=Sn[0:8])

        with (
            tc.tile_pool(name="psumq", bufs=2, space="PSUM") as psumq,
            tc.tile_pool(name="psumc", bufs=1, space="PSUM") as psumc,
        ):
            pcost = psumc.tile([1, BS], F32)
            for c in range(n_chunks):
                chunk = chpool.tile([128, 128], F32, tag="chunk")
                src = d_traj[
                    STATES_PER_CHUNK * c : STATES_PER_CHUNK * (c + 1)
                ].rearrange("a b c -> (a b) c")
                nc.sync.dma_start(out=chunk[:], in_=src)
                pq = psumq.tile([128, 128], F32, tag="pq")
                nc.tensor.matmul(pq[:], bd[:], chunk[:], start=True, stop=True)
                tp = tpool.tile([128, 128], F32, tag="tp")
                nc.vector.tensor_mul(tp[:], chunk[:], pq[:])
                nc.tensor.matmul(
                    pcost[:],
                    ones_col[:],
                    tp[:],
                    start=(c == 0),
                    stop=(c == n_chunks - 1),
                    skip_group_check=True,
                )
            nc.scalar.copy(out=cost_sb[:], in_=pcost[:])
            nc.sync.dma_start(out=d_cost[:], in_=cost_sb[:])

    nc.compile()
    return nc


def make_host_constants(inputs, W1, b1, W2, b2, K, Q, R, Ts):
    """Precompute the per-core input tensors (all float32 numpy)."""
    inputs = np.asarray(inputs, np.float32)
    W1 = np.asarray(W1, np.float32)
    b1 = np.asarray(b1, np.float32)
    W2 = np.asarray(W2, np.float32)
    b2 = np.asarray(b2, np.float32)
    K = np.asarray(K, np.float32)
    Q = np.asarray(Q, np.float32)
    R = np.asarray(R, np.float32)

    x = inputs[:, :SD]  # scaling factors are [1, 1]
    controller = np.concatenate(
        [np.broadcast_to(K[0, : DIM_X - SD], (B, DIM_X - SD)), x], axis=1
    ).astype(np.float32)  # [B, 4]
    z0 = np.array([4.0, 0.0, 0.1, -0.01], np.float32)

    w1a0 = np.empty((9, 128), np.float32)
    w1a1 = np.empty((9, 128), np.float32)
    for half, w in ((0, w1a0), (1, w1a1)):
        cols = slice(128 * half, 128 * (half + 1))
        w[0:4] = W1[0:4, cols]
        w[4:8] = np.broadcast_to(W1[4, cols], (4, 128))
        w[8] = b1[cols]

    w2d0 = np.concatenate([W2[0:128], W2[0:128]], axis=1).astype(np.float32)
    w2d1 = np.concatenate([W2[128:256], W2[128:256]], axis=1).astype(np.float32)
    b2d = np.concatenate([b2, b2])[:, None].astype(np.float32)  # [8,1]

    inv = np.float32(1.0 / (Ts + 1))
    m8 = np.zeros((8, 8), np.float32)
    m8[0:4, 0:4] = (Q + Q.T) * (0.5 * inv)
    m8[4:8, 4:8] = R[0, 0] * inv
    bd = np.zeros((128, 128), np.float32)
    for s in range(STATES_PER_CHUNK):
        bd[8 * s : 8 * (s + 1), 8 * s : 8 * (s + 1)] = m8

    shared = {
        "W1a0": w1a0, "W1a1": w1a1,
        "W2d0": w2d0, "W2d1": w2d1, "b2d": b2d, "BD": bd,
    }
    in_maps = []
    for c in range(N_CORES):
        sl = slice(BS * c, BS * (c + 1))
        nctrl = -controller[sl].T.copy()  # [4, 128]
        cmat = np.concatenate(
            [np.ones((4, BS), np.float32), nctrl], axis=0
        )  # [8, 128]
        sinit = np.empty((9, BS), np.float32)
        sinit[0:4] = z0[:, None]
        sinit[4:8] = nctrl * z0[:, None]
        sinit[8] = 1.0
        in_maps.append(dict(shared, Cmat=cmat, Sinit=sinit))
    return in_maps


_program_cache = {}


def _get_program(n_steps):
    if n_steps not in _program_cache:
        _program_cache[n_steps] = build_program(n_steps)
    return _program_cache[n_steps]


def run(inputs, W1, b1, W2, b2, K, Q, R, Ts, trace=False):
    Ts = int(Ts)
    nc = _get_program(Ts)
    in_maps = make_host_constants(inputs, W1, b1, W2, b2, K, Q, R, Ts)
    res = run_bass_kernel_spmd(nc, in_maps, list(range(N_CORES)), trace=trace)
    out = np.empty((B,), np.float32)
    for c in range(N_CORES):
        out[BS * c : BS * (c + 1)] = np.asarray(res.results[c]["cost"])[0]
    return out, res.exec_time_ns


def kernel(**inputs) -> np.ndarray:
    return run(**inputs)[0]


# revision 15
# speedup vs baseline: 1.1688x; 1.1688x over previous
"""Trainium2 Bass kernel: batched 1000-step controller rollout + LQR-style cost.

Problem: B=1024 independent controllers, each rolling z_{t+1} = MLP([z_t, u_t])
for Ts=1000 steps with u_t = -controller . z_t, then
cost_b = mean_t (z^T Q z + R u^2).

Sharding: data-parallel over batch; 128 controllers per core on 8 cores.

State S[9,128] = [z(4); y(4); ones(1)] with y_k = -ctrl_k * z_k, so
u = sum_k y_k is folded into the layer-1 contraction and b1 rides the ones row
(lhsT rows = [W1[0:4]; W1[4] x4; b1]). All math fp32 (bf16 state diverges over
1000 steps; float32r matmuls are numerically broken on TRN2). fp32 weight
loads stream at a fixed ~1.2GHz (2 passes x M columns), so each h-matmul pays
a ~213ns LDWEIGHTS toll — a single 128-wide group pays it once per half per
step (two 64-wide chains would pay it twice). Latency is hidden by splitting
the post-h pipeline per hidden half: tanh(h0) and the first z-matmul run while
mm_h1 / tanh(h1) are still in flight (separate PSUM banks per half make the
dependencies unambiguous).

Per step:
  psum_h{0,1}  = W1aug_half^T @ S             (2 matmuls, K=9, N=128)
  h{0,1}       = tanh(psum_h{0,1})            (2 ACT ops, bias-free)
  psum_z[8,128]= W2dup_0^T @ h0 + W2dup_1^T @ h1  (2 matmuls, K=128, M=8: W2
                                               cols duplicated so psum_z = [z;z])
  S'[0:8] = (psum_z + b2dup) * [ones(4); negctrl]   (1 DVE scalar_tensor_tensor)
  DMA S'[0:8] -> DRAM trajectory bounce       (off critical path)

Cost pass: DMA 16-state chunks back as [128,128] tiles; q = BD^T @ chunk with
BD block-diag([sym(Q); R*ones(4,4)])/(Ts+1); tp = chunk * q (DVE); accumulate
ones^T @ tp into psum_cost[1,128] across chunks.
"""

import numpy as np
from contextlib import ExitStack

import concourse.bass as bass
import concourse.bacc as bacc
import concourse.tile as tile
from concourse import mybir
from concourse._compat import get_trn_type
from concourse.bass_utils import run_bass_kernel_spmd

F32 = mybir.dt.float32
AFT = mybir.ActivationFunctionType
ALU = mybir.AluOpType

B = 1024
H = 256
SD = 2
DIM_X = 4
N_CORES = 8
BS = B // N_CORES   # 128 batch columns per core
N_SBUF = 3           # persistent S tiles (rotation depth)
STATES_PER_CHUNK = 16


def build_program(n_steps: int):
    """Build the Bass/Tile program for one core (SPMD across 8)."""
    n_states = n_steps + 1
    n_chunks = (n_states + STATES_PER_CHUNK - 1) // STATES_PER_CHUNK
    n_slots = n_chunks * STATES_PER_CHUNK

    nc = bacc.Bacc(
        get_trn_type() or "TRN2",
        target_bir_lowering=False,
        debug=False,
        num_devices=N_CORES,
    )

    d_w1a0 = nc.dram_tensor("W1a0", [9, 128], F32, kind="ExternalInput")
    d_w1a1 = nc.dram_tensor("W1a1", [9, 128], F32, kind="ExternalInput")
    d_w2d0 = nc.dram_tensor("W2d0", [128, 8], F32, kind="ExternalInput")
    d_w2d1 = nc.dram_tensor("W2d1", [128, 8], F32, kind="ExternalInput")
    d_b2d = nc.dram_tensor("b2d", [8, 1], F32, kind="ExternalInput")
    d_cmat = nc.dram_tensor("Cmat", [8, BS], F32, kind="ExternalInput")
    d_sinit = nc.dram_tensor("Sinit", [9, BS], F32, kind="ExternalInput")
    d_bd = nc.dram_tensor("BD", [128, 128], F32, kind="ExternalInput")
    d_cost = nc.dram_tensor("cost", [1, BS], F32, kind="ExternalOutput")
    d_traj = nc.dram_tensor("trajb", [n_slots, 8, BS], F32)  # Internal bounce

    with tile.TileContext(nc) as tc, ExitStack() as ctx:
        consts = ctx.enter_context(tc.tile_pool(name="consts", bufs=1))
        hpool = ctx.enter_context(tc.tile_pool(name="hpool", bufs=2))
        chpool = ctx.enter_context(tc.tile_pool(name="chpool", bufs=3))
        tpool = ctx.enter_context(tc.tile_pool(name="tpool", bufs=3))

        w1a0 = consts.tile([9, 128], F32)
        w1a1 = consts.tile([9, 128], F32)
        w2d0 = consts.tile([128, 8], F32)
        w2d1 = consts.tile([128, 8], F32)
        b2d = consts.tile([8, 1], F32)
        cmat = consts.tile([8, BS], F32)
        bd = consts.tile([128, 128], F32)
        ones_col = consts.tile([128, 1], F32)
        zpad = consts.tile([8, BS], F32)
        cost_sb = consts.tile([1, BS], F32)

        nc.sync.dma_start(out=w1a0[:], in_=d_w1a0[:])
        nc.sync.dma_start(out=w1a1[:], in_=d_w1a1[:])
        nc.sync.dma_start(out=w2d0[:], in_=d_w2d0[:])
        nc.sync.dma_start(out=w2d1[:], in_=d_w2d1[:])
        nc.sync.dma_start(out=b2d[:], in_=d_b2d[:])
        nc.sync.dma_start(out=cmat[:], in_=d_cmat[:])
        nc.sync.dma_start(out=bd[:], in_=d_bd[:])
        nc.vector.memset(ones_col[:], 1.0)
        nc.vector.memset(zpad[:], 0.0)

        # zero the padding slots of the trajectory so they contribute 0 cost
        for i in range(n_states, n_slots):
            nc.sync.dma_start(out=d_traj[i], in_=zpad[:])

        # persistent state tiles; row 8 stays = 1.0 (written by the init DMA,
        # never touched by the per-step state write to rows 0:8)
        S_tiles = [
            consts.tile([9, BS], F32, name=f"S_{k}", tag=f"S_{k}")
            for k in range(N_SBUF)
        ]
        for k in range(N_SBUF):
            nc.sync.dma_start(out=S_tiles[k][:], in_=d_sinit[:])
        nc.sync.dma_start(out=d_traj[0], in_=S_tiles[0][0:8])

        with (
            tc.tile_pool(name="psumh0", bufs=2, space="PSUM") as psumh0,
            tc.tile_pool(name="psumh1", bufs=2, space="PSUM") as psumh1,
            tc.tile_pool(name="psumz", bufs=2, space="PSUM") as psumz,
        ):
            for t in range(n_steps):
                S = S_tiles[t % N_SBUF]
                Sn = S_tiles[(t + 1) % N_SBUF]
                ph0 = psumh0.tile([128, BS], F32, tag="ph0")
                ph1 = psumh1.tile([128, BS], F32, tag="ph1")
                nc.tensor.matmul(ph0[:], w1a0[:], S[:], start=True, stop=True)
                nc.tensor.matmul(ph1[:], w1a1[:], S[:], start=True, stop=True)
                h0 = hpool.tile([128, BS], F32, tag="h0")
                h1 = hpool.tile([128, BS], F32, tag="h1")
                nc.scalar.activation(h0[:], ph0[:], AFT.Tanh)
                nc.scalar.activation(h1[:], ph1[:], AFT.Tanh)
                pz = psumz.tile([8, BS], F32, tag="pz")
                nc.tensor.matmul(pz[:], w2d0[:], h0[:], start=True, stop=False)
                nc.tensor.matmul(pz[:], w2d1[:], h1[:], start=False, stop=True)
                nc.vector.scalar_tensor_tensor(
                    out=Sn[0:8], in0=pz[:], scalar=b2d[:], in1=cmat[:],
                    op0=ALU.add, op1=ALU.mult,
                )
                nc.sync.dma_start(out=d_traj[t + 1], in_=Sn[0:8])

        with (
            tc.tile_pool(name="psumq", bufs=2, space="PSUM") as psumq,
            tc.tile_pool(name="psumc", bufs=1, space="PSUM") as psumc,
        ):
            pcost = psumc.tile([1, BS], F32)
            for c in range(n_chunks):
                chunk = chpool.tile([128, 128], F32, tag="chunk")
                src = d_traj[
                    STATES_PER_CHUNK * c : STATES_PER_CHUNK * (c + 1)
                ].rearrange("a b c -> (a b) c")
                nc.sync.dma_start(out=chunk[:], in_=src)
                pq = psumq.tile([128, 128], F32, tag="pq")
                nc.tensor.matmul(pq[:], bd[:], chunk[:], start=True, stop=True)
                tp = tpool.tile([128, 128], F32, tag="tp")
                nc.vector.tensor_mul(tp[:], chunk[:], pq[:])
                nc.tensor.matmul(
                    pcost[:],
                    ones_col[:],
                    tp[:],
                    start=(c == 0),
                    stop=(c == n_chunks - 1),
                    skip_group_check=True,
                )
            nc.scalar.copy(out=cost_sb[:], in_=pcost[:])
            nc.sync.dma_start(out=d_cost[:], in_=cost_sb[:])

    nc.compile()
    return nc


def make_host_constants(inputs, W1, b1, W2, b2, K, Q, R, Ts):
    """Precompute the per-core input tensors (all float32 numpy)."""
    inputs = np.asarray(inputs, np.float32)
    W1 = np.asarray(W1, np.float32)
    b1 = np.asarray(b1, np.float32)
    W2 = np.asarray(W2, np.float32)
    b2 = np.asarray(b2, np.float32)
    K = np.asarray(K, np.float32)
    Q = np.asarray(Q, np.float32)
    R = np.asarray(R, np.float32)

    x = inputs[:, :SD]  # scaling factors are [1, 1]
    controller = np.concatenate(
        [np.broadcast_to(K[0, : DIM_X - SD], (B, DIM_X - SD)), x], axis=1
    ).astype(np.float32)  # [B, 4]
    z0 = np.array([4.0, 0.0, 0.1, -0.01], np.float32)

    w1a0 = np.empty((9, 128), np.float32)
    w1a1 = np.empty((9, 128), np.float32)
    for half, w in ((0, w1a0), (1, w1a1)):
        cols = slice(128 * half, 128 * (half + 1))
        w[0:4] = W1[0:4, cols]
        w[4:8] = np.broadcast_to(W1[4, cols], (4, 128))
        w[8] = b1[cols]

    w2d0 = np.concatenate([W2[0:128], W2[0:128]], axis=1).astype(np.float32)
    w2d1 = np.concatenate([W2[128:256], W2[128:256]], axis=1).astype(np.float32)
    b2d = np.concatenate([b2, b2])[:, None].astype(np.float32)  # [8,1]

    inv = np.float32(1.0 / (Ts + 1))
    m8 = np.zeros((8, 8), np.float32)
    m8[0:4, 0:4] = (Q + Q.T) * (0.5 * inv)
    m8[4:8, 4:8] = R[0, 0] * inv
    bd = np.zeros((128, 128), np.float32)
    for s in range(STATES_PER_CHUNK):
        bd[8 * s : 8 * (s + 1), 8 * s : 8 * (s + 1)] = m8

    shared = {
        "W1a0": w1a0, "W1a1": w1a1,
        "W2d0": w2d0, "W2d1": w2d1, "b2d": b2d, "BD": bd,
    }
    in_maps = []
    for c in range(N_CORES):
        sl = slice(BS * c, BS * (c + 1))
        nctrl = -controller[sl].T.copy()  # [4, 128]
        cmat = np.concatenate(
            [np.ones((4, BS), np.float32), nctrl], axis=0
        )  # [8, 128]
        sinit = np.empty((9, BS), np.float32)
        sinit[0:4] = z0[:, None]
        sinit[4:8] = nctrl * z0[:, None]
        sinit[8] = 1.0
        in_maps.append(dict(shared, Cmat=cmat, Sinit=sinit))
    return in_maps


_program_cache = {}


def _get_program(n_steps):
    if n_steps not in _program_cache:
        _program_cache[n_steps] = build_program(n_steps)
    return _program_cache[n_steps]


def run(inputs, W1, b1, W2, b2, K, Q, R, Ts, trace=False):
    Ts = int(Ts)
    nc = _get_program(Ts)
    in_maps = make_host_constants(inputs, W1, b1, W2, b2, K, Q, R, Ts)
    res = run_bass_kernel_spmd(nc, in_maps, list(range(N_CORES)), trace=trace)
    out = np.empty((B,), np.float32)
    for c in range(N_CORES):
        out[BS * c : BS * (c + 1)] = np.asarray(res.results[c]["cost"])[0]
    return out, res.exec_time_ns


def kernel(**inputs) -> np.ndarray:
    return run(**inputs)[0]


# revision 16
# speedup vs baseline: 1.1963x; 1.0235x over previous
"""Trainium2 Bass kernel: batched 1000-step controller rollout + LQR-style cost.

Problem: B=1024 independent controllers, each rolling z_{t+1} = MLP([z_t, u_t])
for Ts=1000 steps with u_t = -controller . z_t, then
cost_b = mean_t (z^T Q z + R u^2).

Sharding: data-parallel over batch; 128 controllers per core on 8 cores.

State S[9,128] = [z(4); y(4); ones(1)] with y_k = -ctrl_k * z_k, so
u = sum_k y_k is folded into the layer-1 contraction and b1 rides the ones row
(lhsT rows = [W1[0:4]; W1[4] x4; b1]). All math fp32 (bf16 state diverges over
1000 steps; float32r matmuls are numerically broken on TRN2). fp32 weight
loads stream at a fixed ~1.2GHz (2 passes x M columns), so each h-matmul pays
a ~213ns LDWEIGHTS toll — a single 128-wide group pays it once per half per
step (two 64-wide chains would pay it twice). Latency is hidden by splitting
the post-h pipeline per hidden half: tanh(h0) and the first z-matmul run while
mm_h1 / tanh(h1) are still in flight (separate PSUM banks per half make the
dependencies unambiguous).

Per step:
  psum_h{0,1}  = W1aug_half^T @ S             (2 matmuls, K=9, N=128)
  h{0,1}       = tanh(psum_h{0,1})            (2 ACT ops, bias-free)
  psum_z[8,128]= W2dup_0^T @ h0 + W2dup_1^T @ h1  (2 matmuls, K=128, M=8: W2
                                               cols duplicated so psum_z = [z;z])
  S'[0:8] = (psum_z + b2dup) * [ones(4); negctrl]   (1 DVE scalar_tensor_tensor)
  DMA S'[0:8] -> DRAM trajectory bounce       (off critical path)

Cost pass: DMA 16-state chunks back as [128,128] tiles; q = BD^T @ chunk with
BD block-diag([sym(Q); R*ones(4,4)])/(Ts+1); tp = chunk * q (DVE); accumulate
ones^T @ tp into psum_cost[1,128] across chunks.
"""

import numpy as np
from contextlib import ExitStack

import concourse.bass as bass
import concourse.bacc as bacc
import concourse.tile as tile
from concourse import mybir
from concourse._compat import get_trn_type
from concourse.bass_utils import run_bass_kernel_spmd

F32 = mybir.dt.float32
AFT = mybir.ActivationFunctionType
ALU = mybir.AluOpType

B = 1024
H = 256
SD = 2
DIM_X = 4
N_CORES = 8
BS = B // N_CORES   # 128 batch columns per core
N_SBUF = 3           # persistent S tiles (rotation depth)
STATES_PER_CHUNK = 16


def build_program(n_steps: int):
    """Build the Bass/Tile program for one core (SPMD across 8)."""
    n_states = n_steps + 1
    n_chunks = (n_states + STATES_PER_CHUNK - 1) // STATES_PER_CHUNK
    n_slots = n_chunks * STATES_PER_CHUNK

    nc = bacc.Bacc(
        get_trn_type() or "TRN2",
        target_bir_lowering=False,
        debug=False,
        num_devices=N_CORES,
    )

    d_w1a0 = nc.dram_tensor("W1a0", [9, 128], F32, kind="ExternalInput")
    d_w1a1 = nc.dram_tensor("W1a1", [9, 128], F32, kind="ExternalInput")
    d_w2d0 = nc.dram_tensor("W2d0", [128, 8], F32, kind="ExternalInput")
    d_w2d1 = nc.dram_tensor("W2d1", [128, 8], F32, kind="ExternalInput")
    d_b2d = nc.dram_tensor("b2d", [8, 1], F32, kind="ExternalInput")
    d_cmat = nc.dram_tensor("Cmat", [8, BS], F32, kind="ExternalInput")
    d_sinit = nc.dram_tensor("Sinit", [9, BS], F32, kind="ExternalInput")
    d_bd = nc.dram_tensor("BD", [128, 128], F32, kind="ExternalInput")
    d_cost = nc.dram_tensor("cost", [1, BS], F32, kind="ExternalOutput")
    d_traj = nc.dram_tensor("trajb", [n_slots, 8, BS], F32)  # Internal bounce

    with tile.TileContext(nc) as tc, ExitStack() as ctx:
        consts = ctx.enter_context(tc.tile_pool(name="consts", bufs=1))
        hpool = ctx.enter_context(tc.tile_pool(name="hpool", bufs=2))
        chpool = ctx.enter_context(tc.tile_pool(name="chpool", bufs=3))
        tpool = ctx.enter_context(tc.tile_pool(name="tpool", bufs=3))

        w1a0 = consts.tile([9, 128], F32)
        w1a1 = consts.tile([9, 128], F32)
        w2d0 = consts.tile([128, 8], F32)
        w2d1 = consts.tile([128, 8], F32)
        b2d = consts.tile([8, 1], F32)
        cmat = consts.tile([8, BS], F32)
        bd = consts.tile([128, 128], F32)
        ones_col = consts.tile([128, 1], F32)
        zpad = consts.tile([8, BS], F32)
        cost_sb = consts.tile([1, BS], F32)

        nc.sync.dma_start(out=w1a0[:], in_=d_w1a0[:])
        nc.sync.dma_start(out=w1a1[:], in_=d_w1a1[:])
        nc.sync.dma_start(out=w2d0[:], in_=d_w2d0[:])
        nc.sync.dma_start(out=w2d1[:], in_=d_w2d1[:])
        nc.sync.dma_start(out=b2d[:], in_=d_b2d[:])
        nc.sync.dma_start(out=cmat[:], in_=d_cmat[:])
        nc.sync.dma_start(out=bd[:], in_=d_bd[:])
        nc.vector.memset(ones_col[:], 1.0)
        nc.vector.memset(zpad[:], 0.0)

        # zero the padding slots of the trajectory so they contribute 0 cost
        for i in range(n_states, n_slots):
            nc.sync.dma_start(out=d_traj[i], in_=zpad[:])

        # persistent state tiles; row 8 stays = 1.0 (written by the init DMA,
        # never touched by the per-step state write to rows 0:8)
        S_tiles = [
            consts.tile([9, BS], F32, name=f"S_{k}", tag=f"S_{k}")
            for k in range(N_SBUF)
        ]
        for k in range(N_SBUF):
            nc.sync.dma_start(out=S_tiles[k][:], in_=d_sinit[:])
        nc.sync.dma_start(out=d_traj[0], in_=S_tiles[0][0:8])

        HB = BS // 2  # batch half for the z/stt tail pipeline
        with (
            tc.tile_pool(name="psumh0", bufs=2, space="PSUM") as psumh0,
            tc.tile_pool(name="psumh1", bufs=2, space="PSUM") as psumh1,
            tc.tile_pool(name="psumzL", bufs=2, space="PSUM") as psumzL,
            tc.tile_pool(name="psumzR", bufs=2, space="PSUM") as psumzR,
        ):
            for t in range(n_steps):
                S = S_tiles[t % N_SBUF]
                Sn = S_tiles[(t + 1) % N_SBUF]
                ph0 = psumh0.tile([128, BS], F32, tag="ph0")
                ph1 = psumh1.tile([128, BS], F32, tag="ph1")
                nc.tensor.matmul(ph0[:], w1a0[:], S[:], start=True, stop=True)
                nc.tensor.matmul(ph1[:], w1a1[:], S[:], start=True, stop=True)
                h0 = hpool.tile([128, BS], F32, tag="h0")
                h1 = hpool.tile([128, BS], F32, tag="h1")
                nc.scalar.activation(h0[:], ph0[:], AFT.Tanh)
                nc.scalar.activation(h1[:], ph1[:], AFT.Tanh)
                # z-layer split by batch half: the left half's state update
                # (DVE) overlaps the right half's z-matmuls (PE)
                pzL = psumzL.tile([8, HB], F32, tag="pzL")
                pzR = psumzR.tile([8, HB], F32, tag="pzR")
                nc.tensor.matmul(
                    pzL[:], w2d0[:], h0[:, 0:HB], start=True, stop=False
                )
                nc.tensor.matmul(
                    pzL[:], w2d1[:], h1[:, 0:HB], start=False, stop=True
                )
                nc.vector.scalar_tensor_tensor(
                    out=Sn[0:8, 0:HB], in0=pzL[:], scalar=b2d[:],
                    in1=cmat[:, 0:HB], op0=ALU.add, op1=ALU.mult,
                )
                nc.tensor.matmul(
                    pzR[:], w2d0[:], h0[:, HB:BS], start=True, stop=False
                )
                nc.tensor.matmul(
                    pzR[:], w2d1[:], h1[:, HB:BS], start=False, stop=True
                )
                nc.vector.scalar_tensor_tensor(
                    out=Sn[0:8, HB:BS], in0=pzR[:], scalar=b2d[:],
                    in1=cmat[:, HB:BS], op0=ALU.add, op1=ALU.mult,
                )
                nc.sync.dma_start(out=d_traj[t + 1], in_=Sn[0:8])

        with (
            tc.tile_pool(name="psumq", bufs=2, space="PSUM") as psumq,
            tc.tile_pool(name="psumc", bufs=1, space="PSUM") as psumc,
        ):
            pcost = psumc.tile([1, BS], F32)
            for c in range(n_chunks):
                chunk = chpool.tile([128, 128], F32, tag="chunk")
                src = d_traj[
                    STATES_PER_CHUNK * c : STATES_PER_CHUNK * (c + 1)
                ].rearrange("a b c -> (a b) c")
                nc.sync.dma_start(out=chunk[:], in_=src)
                pq = psumq.tile([128, 128], F32, tag="pq")
                nc.tensor.matmul(pq[:], bd[:], chunk[:], start=True, stop=True)
                tp = tpool.tile([128, 128], F32, tag="tp")
                nc.vector.tensor_mul(tp[:], chunk[:], pq[:])
                nc.tensor.matmul(
                    pcost[:],
                    ones_col[:],
                    tp[:],
                    start=(c == 0),
                    stop=(c == n_chunks - 1),
                    skip_group_check=True,
                )
            nc.scalar.copy(out=cost_sb[:], in_=pcost[:])
            nc.sync.dma_start(out=d_cost[:], in_=cost_sb[:])

    nc.compile()
    return nc


def make_host_constants(inputs, W1, b1, W2, b2, K, Q, R, Ts):
    """Precompute the per-core input tensors (all float32 numpy)."""
    inputs = np.asarray(inputs, np.float32)
    W1 = np.asarray(W1, np.float32)
    b1 = np.asarray(b1, np.float32)
    W2 = np.asarray(W2, np.float32)
    b2 = np.asarray(b2, np.float32)
    K = np.asarray(K, np.float32)
    Q = np.asarray(Q, np.float32)
    R = np.asarray(R, np.float32)

    x = inputs[:, :SD]  # scaling factors are [1, 1]
    controller = np.concatenate(
        [np.broadcast_to(K[0, : DIM_X - SD], (B, DIM_X - SD)), x], axis=1
    ).astype(np.float32)  # [B, 4]
    z0 = np.array([4.0, 0.0, 0.1, -0.01], np.float32)

    w1a0 = np.empty((9, 128), np.float32)
    w1a1 = np.empty((9, 128), np.float32)
    for half, w in ((0, w1a0), (1, w1a1)):
        cols = slice(128 * half, 128 * (half + 1))
        w[0:4] = W1[0:4, cols]
        w[4:8] = np.broadcast_to(W1[4, cols], (4, 128))
        w[8] = b1[cols]

    w2d0 = np.concatenate([W2[0:128], W2[0:128]], axis=1).astype(np.float32)
    w2d1 = np.concatenate([W2[128:256], W2[128:256]], axis=1).astype(np.float32)
    b2d = np.concatenate([b2, b2])[:, None].astype(np.float32)  # [8,1]

    inv = np.float32(1.0 / (Ts + 1))
    m8 = np.zeros((8, 8), np.float32)
    m8[0:4, 0:4] = (Q + Q.T) * (0.5 * inv)
    m8[4:8, 4:8] = R[0, 0] * inv
    bd = np.zeros((128, 128), np.float32)
    for s in range(STATES_PER_CHUNK):
        bd[8 * s : 8 * (s + 1), 8 * s : 8 * (s + 1)] = m8

    shared = {
        "W1a0": w1a0, "W1a1": w1a1,
        "W2d0": w2d0, "W2d1": w2d1, "b2d": b2d, "BD": bd,
    }
    in_maps = []
    for c in range(N_CORES):
        sl = slice(BS * c, BS * (c + 1))
        nctrl = -controller[sl].T.copy()  # [4, 128]
        cmat = np.concatenate(
            [np.ones((4, BS), np.float32), nctrl], axis=0
        )  # [8, 128]
        sinit = np.empty((9, BS), np.float32)
        sinit[0:4] = z0[:, None]
        sinit[4:8] = nctrl * z0[:, None]
        sinit[8] = 1.0
        in_maps.append(dict(shared, Cmat=cmat, Sinit=sinit))
    return in_maps


_program_cache = {}


def _get_program(n_steps):
    if n_steps not in _program_cache:
        _program_cache[n_steps] = build_program(n_steps)
    return _program_cache[n_steps]


def run(inputs, W1, b1, W2, b2, K, Q, R, Ts, trace=False):
    Ts = int(Ts)
    nc = _get_program(Ts)
    in_maps = make_host_constants(inputs, W1, b1, W2, b2, K, Q, R, Ts)
    res = run_bass_kernel_spmd(nc, in_maps, list(range(N_CORES)), trace=trace)
    out = np.empty((B,), np.float32)
    for c in range(N_CORES):
        out[BS * c : BS * (c + 1)] = np.asarray(res.results[c]["cost"])[0]
    return out, res.exec_time_ns


def kernel(**inputs) -> np.ndarray:
    return run(**inputs)[0]
